# revision 33
# baseline (speedup 1.0000x reference)
"""Trainium2 Bass kernel for nn_DataEmbedding (token conv-embed + sinusoid tables).

Math: out[b, d, w] = tok[b, d, w] + pe[d, w] + temp[b, d, w]
  tok: 74 depthwise stride-3 ksize-8 convs over x[b, :, c], laid out d = n*21+c
       (d=1533 is kernel 73 applied to channel 0), w = window index (516).
  temp: hour/weekday/day/month table gathers (indices all in 0..3).

Device scheme (pure data-parallel, 8 batches per core):
  Conv taps decompose as k = 3a+r. With R'[32r+c, m] = x[b, 3(m-7)+r, c]
  (zeros for m<7), window w of tap (a, r) reads R'[32r+c, w+a], so the whole
  module collapses into two PSUM-accumulated fp32r matmuls per output tile:
   pass A (K=128): rows 0..95 R' (shift 0) vs lhs0, rows 96..111 the 16-row
     combined sinusoid table vs a one-hot built on-device from x_mark (this
     folds the 4 embedding-table gathers into the same matmul), rows
     112..127 a shift-1 copy of R' rows 0..15 vs lhs1[rows 0..15].
   pass B (K=110): remaining shift-1 rows vs lhs1-rest, shift-2 vs lhs2.
  pe is added during the PSUM->SBUF eviction (DVE tensor_tensor add).

  R' is built on-chip without element-granular DMA: x[b] is DMA'd
  contiguously into a [32, 48*32] tile (s mod 32 on partitions), one DVE
  32x32 block-transpose turns it into XT[c, s], and three strided
  tensor_copy ops (free-dim stride 3) deinterleave XT into R'.
"""

import sys
import numpy as np

sys.path.insert(0, "/opt/trn_rl_repo")

B, S, C, D = 64, 1534, 21, 516
NCORES = 8
PB = B // NCORES          # batches per core
M2 = 1536                 # padded d-dim (12 x 128)
BW = 520                  # per-batch column block in RHX/RHY
NQ = 12                   # M-chunks of 128
NH = 2                    # N-chunks of 258
NW = 258
KB = 110                  # pass-B contraction rows
TCOLS = [3, 2, 1, 0]      # x_mark column for table t (hour, weekday, day, month)

_CACHE = {}


def _build_module():
    import concourse.bacc as bacc
    import concourse.mybir as mybir
    import concourse.tile as tile

    f32 = mybir.dt.float32
    f32r = mybir.dt.float32r

    nc = bacc.Bacc("TRN2", target_bir_lowering=False, debug=False,
                   num_devices=NCORES)

    xs = nc.dram_tensor("xs", [PB, S, C], f32, kind="ExternalInput")
    xm16 = nc.dram_tensor("xm16", [PB, 16, S], f32, kind="ExternalInput")
    lhsa = nc.dram_tensor("lhsa", [128, M2], f32r, kind="ExternalInput")
    lhsb = nc.dram_tensor("lhsb", [KB, M2], f32r, kind="ExternalInput")
    tt = nc.dram_tensor("tt", [16, D], f32r, kind="ExternalInput")
    vj = nc.dram_tensor("vj", [16, 1], f32, kind="ExternalInput")
    pe_in = nc.dram_tensor("pe_in", [S, D], f32, kind="ExternalInput")
    out = nc.dram_tensor("out", [PB, S, D], f32, kind="ExternalOutput")

    with tile.TileContext(nc) as tc:
        with (
            tc.tile_pool(name="consts", bufs=1) as consts,
            tc.tile_pool(name="xnp", bufs=2) as xnp,
            tc.tile_pool(name="xtp", bufs=2) as xtp,
            tc.tile_pool(name="xmp", bufs=2) as xmp,
            tc.tile_pool(name="outp", bufs=3) as outp,
            tc.tile_pool(name="psum", bufs=8, space="PSUM") as psum,
        ):
            LHS_A = consts.tile([128, PB * M2], f32r)
            LHS_B = consts.tile([KB, M2], f32r)
            RHX = consts.tile([128, PB * BW], f32r)
            RHY = consts.tile([KB, PB * BW], f32r)
            PE_SB = consts.tile([128, NQ * D], f32)
            ZT = consts.tile([128, BW], f32)
            VJS = consts.tile([16, 1], f32)

            nc.vector.memset(PE_SB[96:128, (NQ - 1) * D:NQ * D], 0.0)
            nc.gpsimd.memset(ZT[:], 0.0)
            nc.scalar.dma_start(LHS_B[:], lhsb[:])
            nc.scalar.dma_start(VJS[:], vj[:])
            nc.scalar.dma_start(
                PE_SB[:, 0:11 * D].rearrange("p (q w) -> p q w", w=D),
                pe_in[0:11 * 128, :].rearrange("(q p) w -> p q w", p=128))
            nc.scalar.dma_start(PE_SB[0:S - 11 * 128, 11 * D:NQ * D],
                                pe_in[11 * 128:S, :])
            # lhsA constant: one HBM load then log2 doubling in SBUF
            nc.scalar.dma_start(LHS_A[:, 0:M2], lhsa[:])
            for sh in range(3):
                n = 1 << sh
                nc.scalar.dma_start(LHS_A[:, n * M2:2 * n * M2],
                                    LHS_A[:, 0:n * M2])
            # zero-fill all RHX blocks from the SBUF zeros tile, then
            # replicate TT (block 0 from HBM, doubling for the rest)
            zv = ZT[:].bitcast(f32r)
            for b in range(PB):
                nc.sync.dma_start(RHX[:, b * BW:(b + 1) * BW], zv)
            nc.scalar.dma_start(RHX[96:112, 0:D], tt[:])
            for sh in range(3):
                n = 1 << sh
                nc.scalar.dma_start(
                    RHX[96:112, n * BW:n * BW + (n - 1) * BW + D],
                    RHX[96:112, 0:(n - 1) * BW + D])

            evict = 0
            for b in range(PB):
                c0 = b * BW
                # load x[b] with s%32 on partitions: Xn[pp, 32j+c] = x[b, 32j+pp, c]
                xn = xnp.tile([32, 1536], f32)
                xnv = xn[:].rearrange("p (j c) -> p j c", c=32)
                # init only the pad lanes the transpose will read
                nc.vector.memset(xnv[:, :, C:32], 0.0)
                nc.vector.memset(xn[0:32, 47 * 32:47 * 32 + C], 0.0)
                nc.sync.dma_start(
                    xnv[:, 0:47, 0:C],
                    xs[b, 0:1504, :].rearrange("(j pp) c -> pp j c", pp=32))
                nc.sync.dma_start(xn[0:30, 47 * 32:47 * 32 + C],
                                  xs[b, 1504:1534, :])
                # one block-transpose => XT[c, s] = x[b, s, c]
                xt = xtp.tile([32, 1536], f32)
                nc.vector.transpose(xt[:], xn[:])
                # stride-3 deinterleave into R' rows 32r+c (f32 -> f32r)
                xtr = xt[0:21, 0:1533].rearrange("p (m r) -> r p m", r=3)
                for r in range(3):
                    nc.scalar.copy(
                        RHX[32 * r:32 * r + 21, c0 + 7:c0 + 7 + 511], xtr[r])

                # one-hot H[b] straight into LHS_A rows 96..111
                xmt = xmp.tile([16, S], f32)
                nc.scalar.dma_start(xmt[:], xm16[b])
                nc.vector.tensor_scalar(
                    LHS_A[96:112, b * M2:b * M2 + S], xmt[:], VJS[:], None,
                    mybir.AluOpType.is_equal)

                # shifted duplicates for this block (SBUF->SBUF)
                nc.scalar.dma_start(RHX[112:128, c0:c0 + 517],
                                    RHX[0:16, c0 + 1:c0 + 518])
                nc.scalar.dma_start(RHY[0:5, c0:c0 + 517],
                                    RHX[16:21, c0 + 1:c0 + 518])
                nc.scalar.dma_start(RHY[5:26, c0:c0 + 517],
                                    RHX[32:53, c0 + 1:c0 + 518])
                nc.scalar.dma_start(RHY[26:47, c0:c0 + 517],
                                    RHX[64:85, c0 + 1:c0 + 518])
                nc.scalar.dma_start(RHY[47:68, c0:c0 + 516],
                                    RHX[0:21, c0 + 2:c0 + 518])
                nc.scalar.dma_start(RHY[68:89, c0:c0 + 516],
                                    RHX[32:53, c0 + 2:c0 + 518])
                nc.scalar.dma_start(RHY[89:110, c0:c0 + 516],
                                    RHX[64:85, c0 + 2:c0 + 518])

                for qg, qn in ((0, 4), (4, 4), (8, 3), (11, 1)):
                    out_t = outp.tile([128, 4 * D], f32, tag="out_t")
                    for qi in range(qn):
                        q = qg + qi
                        for h in range(NH):
                            w0 = h * NW
                            ps = psum.tile([128, NW], f32)
                            nc.tensor.matmul(
                                ps[:],
                                LHS_A[:, b * M2 + q * 128:b * M2 + (q + 1) * 128],
                                RHX[:, b * BW + w0:b * BW + w0 + NW],
                                start=True, stop=False)
                            nc.tensor.matmul(
                                ps[:],
                                LHS_B[:, q * 128:(q + 1) * 128],
                                RHY[:, b * BW + w0:b * BW + w0 + NW],
                                start=False, stop=True)
                            ot = out_t[:, qi * D + w0:qi * D + w0 + NW]
                            pev = PE_SB[:, q * D + w0:q * D + w0 + NW]
                            # spread PSUM eviction across DVE / ACT+GpSimd
                            if evict % 3 == 2:
                                nc.scalar.copy(ot, ps[:])
                                nc.gpsimd.tensor_tensor(
                                    ot, ot, pev, mybir.AluOpType.add)
                            else:
                                nc.vector.tensor_tensor(
                                    ot, ps[:], pev, mybir.AluOpType.add)
                            evict += 1
                    if qg == 11:
                        nc.sync.dma_start(out[b, 11 * 128:S, :],
                                          out_t[0:S - 11 * 128, 0:D])
                    else:
                        nc.sync.dma_start(
                            out[b, qg * 128:(qg + qn) * 128, :]
                            .rearrange("(q p) w -> p q w", p=128),
                            out_t[:, 0:qn * D])

    nc.compile()
    return nc


def _host_prep(inputs):
    x = np.ascontiguousarray(np.asarray(inputs["x"], dtype=np.float32))
    xm = np.asarray(inputs["x_mark"]).astype(np.float32)
    kern = np.asarray(inputs["kernels"], dtype=np.float32)
    pe = np.ascontiguousarray(np.asarray(inputs["pe"], dtype=np.float32))
    tabs = [np.asarray(inputs[k], dtype=np.float32)
            for k in ("hour_tab", "weekday_tab", "day_tab", "month_tab")]

    # xm16[b, 4t+v, s] = x_mark[b, s, TCOLS[t]]  (index rows repeated 4x)
    xm16 = np.ascontiguousarray(
        np.repeat(xm[:, :, TCOLS].transpose(0, 2, 1), 4, axis=1))
    vj = np.tile(np.arange(4, dtype=np.float32), 4)[:, None].copy()
    ttab = np.ascontiguousarray(
        np.concatenate([t[:4] for t in tabs], axis=0).astype(np.float32))

    # conv lhs matrices: row index 32r+c, col n*21+c; col 1533 = kernel 73, c=0
    def build_lhs(a):
        L = np.zeros((96, M2), np.float32)
        n21 = np.arange(73) * 21
        for r in range(3):
            k = 3 * a + r
            if k >= 8:
                continue
            for c in range(C):
                L[32 * r + c, n21 + c] = kern[:73, k]
            L[32 * r, 1533] = kern[73, k]
        return L

    lhs0, lhs1, lhs2 = build_lhs(0), build_lhs(1), build_lhs(2)
    lhsa = np.zeros((128, M2), np.float32)
    lhsa[0:96] = lhs0
    lhsa[112:128] = lhs1[0:16]
    lhsb = np.zeros((KB, M2), np.float32)
    lhsb[0:5] = lhs1[16:21]
    lhsb[5:26] = lhs1[32:53]
    lhsb[26:47] = lhs1[64:85]
    lhsb[47:68] = lhs2[0:21]
    lhsb[68:89] = lhs2[32:53]
    lhsb[89:110] = lhs2[64:85]
    return x, xm16, lhsa, lhsb, ttab, vj, pe


def _run(inputs, trace=False, **kw):
    from concourse.bass_utils import run_bass_kernel_spmd

    if "nc" not in _CACHE:
        _CACHE["nc"] = _build_module()
    nc = _CACHE["nc"]

    x, xm16, lhsa, lhsb, ttab, vj, pe = _host_prep(inputs)
    in_maps = []
    for i in range(NCORES):
        in_maps.append({
            "xs": x[i * PB:(i + 1) * PB],
            "xm16": np.ascontiguousarray(xm16[i * PB:(i + 1) * PB]),
            "lhsa": lhsa, "lhsb": lhsb, "tt": ttab, "vj": vj, "pe_in": pe,
        })
    res = run_bass_kernel_spmd(nc, in_maps, list(range(NCORES)),
                               trace=trace, **kw)
    full = np.concatenate([res.results[i]["out"] for i in range(NCORES)],
                          axis=0)
    return full, res


def kernel(**inputs):
    full, _ = _run(inputs)
    return full


# revision 34
# speedup vs baseline: 1.1043x; 1.1043x over previous
"""Trainium2 Bass kernel for nn_DataEmbedding (token conv-embed + sinusoid tables).

Math: out[b, d, w] = tok[b, d, w] + pe[d, w] + temp[b, d, w]
  tok: 74 depthwise stride-3 ksize-8 convs over x[b, :, c], laid out d = n*21+c
       (d=1533 is kernel 73 applied to channel 0), w = window index (516).
  temp: hour/weekday/day/month table gathers (indices all in 0..3).

Device scheme (pure data-parallel, 8 batches per core):
  Conv taps decompose as k = 3a+r. With R'[32r+c, m] = x[b, 3(m-7)+r, c]
  (zeros for m<7), window w of tap (a, r) reads R'[32r+c, w+a], so the whole
  module collapses into two PSUM-accumulated fp32r matmuls per output tile:
   pass A (K=128): rows 0..95 R' (shift 0) vs lhs0, rows 96..111 the 16-row
     combined sinusoid table vs a one-hot built on-device from x_mark (this
     folds the 4 embedding-table gathers into the same matmul), rows
     112..127 a shift-1 copy of R' rows 0..15 vs lhs1[rows 0..15].
   pass B (K=110): remaining shift-1 rows vs lhs1-rest, shift-2 vs lhs2.
  pe is added during the PSUM->SBUF eviction (DVE tensor_tensor add).

  R' is built on-chip without element-granular DMA: x[b] is DMA'd
  contiguously into a [32, 48*32] tile (s mod 32 on partitions), one DVE
  32x32 block-transpose turns it into XT[c, s], and three strided
  tensor_copy ops (free-dim stride 3) deinterleave XT into R'.
"""

import sys
import numpy as np

sys.path.insert(0, "/opt/trn_rl_repo")

B, S, C, D = 64, 1534, 21, 516
NCORES = 8
PB = B // NCORES          # batches per core
M2 = 1536                 # padded d-dim (12 x 128)
BW = 520                  # per-batch column block in RHX/RHY
NQ = 12                   # M-chunks of 128
NH = 2                    # N-chunks of 258
NW = 258
KB = 110                  # pass-B contraction rows
TCOLS = [3, 2, 1, 0]      # x_mark column for table t (hour, weekday, day, month)

_CACHE = {}


def _build_module():
    import concourse.bacc as bacc
    import concourse.mybir as mybir
    import concourse.tile as tile

    f32 = mybir.dt.float32
    f32r = mybir.dt.float32r

    nc = bacc.Bacc("TRN2", target_bir_lowering=False, debug=False,
                   num_devices=NCORES)

    xs = nc.dram_tensor("xs", [PB, S, C], f32, kind="ExternalInput")
    xm16 = nc.dram_tensor("xm16", [PB, 16, S], f32, kind="ExternalInput")
    lhsa = nc.dram_tensor("lhsa", [128, M2], f32r, kind="ExternalInput")
    lhsb = nc.dram_tensor("lhsb", [KB, M2], f32r, kind="ExternalInput")
    tt = nc.dram_tensor("tt", [16, D], f32r, kind="ExternalInput")
    vj = nc.dram_tensor("vj", [16, 1], f32, kind="ExternalInput")
    pe_in = nc.dram_tensor("pe_in", [S, D], f32, kind="ExternalInput")
    out = nc.dram_tensor("out", [PB, S, D], f32, kind="ExternalOutput")

    with tile.TileContext(nc) as tc:
        with (
            tc.tile_pool(name="consts", bufs=1) as consts,
            tc.tile_pool(name="xnp", bufs=2) as xnp,
            tc.tile_pool(name="xtp", bufs=2) as xtp,
            tc.tile_pool(name="xmp", bufs=2) as xmp,
            tc.tile_pool(name="outp", bufs=3) as outp,
            tc.tile_pool(name="psum", bufs=8, space="PSUM") as psum,
        ):
            LHS_A = consts.tile([128, PB * M2], f32r)
            LHS_B = consts.tile([KB, M2], f32r)
            RHX = consts.tile([128, PB * BW], f32r)
            RHY = consts.tile([KB, PB * BW], f32r)
            PE_SB = consts.tile([128, NQ * D], f32)
            ZT = consts.tile([128, BW], f32)
            VJS = consts.tile([16, 1], f32)

            nc.vector.memset(PE_SB[96:128, (NQ - 1) * D:NQ * D], 0.0)
            # constants ride the (otherwise idle) SWDGE queue so they never
            # block the per-batch critical path on the HWDGE queues
            nc.gpsimd.memset(ZT[:], 0.0)
            nc.gpsimd.dma_start(LHS_A[:, 0:M2], lhsa[:])
            nc.gpsimd.dma_start(VJS[:], vj[:])
            nc.gpsimd.dma_start(LHS_B[:], lhsb[:])
            nc.gpsimd.dma_start(
                PE_SB[:, 0:11 * D].rearrange("p (q w) -> p q w", w=D),
                pe_in[0:11 * 128, :].rearrange("(q p) w -> p q w", p=128))
            nc.gpsimd.dma_start(PE_SB[0:S - 11 * 128, 11 * D:NQ * D],
                                pe_in[11 * 128:S, :])
            for sh in range(3):
                n = 1 << sh
                nc.gpsimd.dma_start(LHS_A[:, n * M2:2 * n * M2],
                                    LHS_A[:, 0:n * M2])
            zv = ZT[:].bitcast(f32r)

            evict = 0
            for b in range(PB):
                c0 = b * BW
                # zero-fill this block, then place TT (HBM for b=0,
                # block-0 copy afterwards)
                nc.sync.dma_start(RHX[:, c0:c0 + BW], zv)
                if b == 0:
                    nc.sync.dma_start(RHX[96:112, 0:D], tt[:])
                else:
                    nc.sync.dma_start(RHX[96:112, c0:c0 + D],
                                      RHX[96:112, 0:D])
                # load x[b] with s%32 on partitions: Xn[pp, 32j+c] = x[b, 32j+pp, c]
                xn = xnp.tile([32, 1536], f32)
                xnv = xn[:].rearrange("p (j c) -> p j c", c=32)
                # init only the pad lanes the transpose will read
                nc.vector.memset(xnv[:, :, C:32], 0.0)
                nc.vector.memset(xn[0:32, 47 * 32:47 * 32 + C], 0.0)
                nc.sync.dma_start(
                    xnv[:, 0:47, 0:C],
                    xs[b, 0:1504, :].rearrange("(j pp) c -> pp j c", pp=32))
                nc.sync.dma_start(xn[0:30, 47 * 32:47 * 32 + C],
                                  xs[b, 1504:1534, :])
                # one block-transpose => XT[c, s] = x[b, s, c]
                xt = xtp.tile([32, 1536], f32)
                nc.vector.transpose(xt[:], xn[:])
                # stride-3 deinterleave into R' rows 32r+c (f32 -> f32r)
                xtr = xt[0:21, 0:1533].rearrange("p (m r) -> r p m", r=3)
                for r in range(3):
                    nc.scalar.copy(
                        RHX[32 * r:32 * r + 21, c0 + 7:c0 + 7 + 511], xtr[r])

                # one-hot H[b] straight into LHS_A rows 96..111
                xmt = xmp.tile([16, S], f32)
                nc.scalar.dma_start(xmt[:], xm16[b])
                nc.vector.tensor_scalar(
                    LHS_A[96:112, b * M2:b * M2 + S], xmt[:], VJS[:], None,
                    mybir.AluOpType.is_equal)

                # shifted duplicates for this block (SBUF->SBUF)
                nc.scalar.dma_start(RHX[112:128, c0:c0 + 517],
                                    RHX[0:16, c0 + 1:c0 + 518])
                nc.scalar.dma_start(RHY[0:5, c0:c0 + 517],
                                    RHX[16:21, c0 + 1:c0 + 518])
                nc.scalar.dma_start(RHY[5:26, c0:c0 + 517],
                                    RHX[32:53, c0 + 1:c0 + 518])
                nc.scalar.dma_start(RHY[26:47, c0:c0 + 517],
                                    RHX[64:85, c0 + 1:c0 + 518])
                nc.scalar.dma_start(RHY[47:68, c0:c0 + 516],
                                    RHX[0:21, c0 + 2:c0 + 518])
                nc.scalar.dma_start(RHY[68:89, c0:c0 + 516],
                                    RHX[32:53, c0 + 2:c0 + 518])
                nc.scalar.dma_start(RHY[89:110, c0:c0 + 516],
                                    RHX[64:85, c0 + 2:c0 + 518])

                for qg, qn in ((0, 4), (4, 4), (8, 3), (11, 1)):
                    out_t = outp.tile([128, 4 * D], f32, tag="out_t")
                    for qi in range(qn):
                        q = qg + qi
                        for h in range(NH):
                            w0 = h * NW
                            ps = psum.tile([128, NW], f32)
                            nc.tensor.matmul(
                                ps[:],
                                LHS_A[:, b * M2 + q * 128:b * M2 + (q + 1) * 128],
                                RHX[:, b * BW + w0:b * BW + w0 + NW],
                                start=True, stop=False)
                            nc.tensor.matmul(
                                ps[:],
                                LHS_B[:, q * 128:(q + 1) * 128],
                                RHY[:, b * BW + w0:b * BW + w0 + NW],
                                start=False, stop=True)
                            ot = out_t[:, qi * D + w0:qi * D + w0 + NW]
                            pev = PE_SB[:, q * D + w0:q * D + w0 + NW]
                            # spread PSUM eviction across DVE / ACT+GpSimd
                            if evict % 3 == 2:
                                nc.scalar.copy(ot, ps[:])
                                nc.gpsimd.tensor_tensor(
                                    ot, ot, pev, mybir.AluOpType.add)
                            else:
                                nc.vector.tensor_tensor(
                                    ot, ps[:], pev, mybir.AluOpType.add)
                            evict += 1
                    if qg == 11:
                        nc.sync.dma_start(out[b, 11 * 128:S, :],
                                          out_t[0:S - 11 * 128, 0:D])
                    else:
                        nc.sync.dma_start(
                            out[b, qg * 128:(qg + qn) * 128, :]
                            .rearrange("(q p) w -> p q w", p=128),
                            out_t[:, 0:qn * D])

    nc.compile()
    return nc


def _host_prep(inputs):
    x = np.ascontiguousarray(np.asarray(inputs["x"], dtype=np.float32))
    xm = np.asarray(inputs["x_mark"]).astype(np.float32)
    kern = np.asarray(inputs["kernels"], dtype=np.float32)
    pe = np.ascontiguousarray(np.asarray(inputs["pe"], dtype=np.float32))
    tabs = [np.asarray(inputs[k], dtype=np.float32)
            for k in ("hour_tab", "weekday_tab", "day_tab", "month_tab")]

    # xm16[b, 4t+v, s] = x_mark[b, s, TCOLS[t]]  (index rows repeated 4x)
    xm16 = np.ascontiguousarray(
        np.repeat(xm[:, :, TCOLS].transpose(0, 2, 1), 4, axis=1))
    vj = np.tile(np.arange(4, dtype=np.float32), 4)[:, None].copy()
    ttab = np.ascontiguousarray(
        np.concatenate([t[:4] for t in tabs], axis=0).astype(np.float32))

    # conv lhs matrices: row index 32r+c, col n*21+c; col 1533 = kernel 73, c=0
    def build_lhs(a):
        L = np.zeros((96, M2), np.float32)
        n21 = np.arange(73) * 21
        for r in range(3):
            k = 3 * a + r
            if k >= 8:
                continue
            for c in range(C):
                L[32 * r + c, n21 + c] = kern[:73, k]
            L[32 * r, 1533] = kern[73, k]
        return L

    lhs0, lhs1, lhs2 = build_lhs(0), build_lhs(1), build_lhs(2)
    lhsa = np.zeros((128, M2), np.float32)
    lhsa[0:96] = lhs0
    lhsa[112:128] = lhs1[0:16]
    lhsb = np.zeros((KB, M2), np.float32)
    lhsb[0:5] = lhs1[16:21]
    lhsb[5:26] = lhs1[32:53]
    lhsb[26:47] = lhs1[64:85]
    lhsb[47:68] = lhs2[0:21]
    lhsb[68:89] = lhs2[32:53]
    lhsb[89:110] = lhs2[64:85]
    return x, xm16, lhsa, lhsb, ttab, vj, pe


def _run(inputs, trace=False, **kw):
    from concourse.bass_utils import run_bass_kernel_spmd

    if "nc" not in _CACHE:
        _CACHE["nc"] = _build_module()
    nc = _CACHE["nc"]

    x, xm16, lhsa, lhsb, ttab, vj, pe = _host_prep(inputs)
    in_maps = []
    for i in range(NCORES):
        in_maps.append({
            "xs": x[i * PB:(i + 1) * PB],
            "xm16": np.ascontiguousarray(xm16[i * PB:(i + 1) * PB]),
            "lhsa": lhsa, "lhsb": lhsb, "tt": ttab, "vj": vj, "pe_in": pe,
        })
    res = run_bass_kernel_spmd(nc, in_maps, list(range(NCORES)),
                               trace=trace, **kw)
    full = np.concatenate([res.results[i]["out"] for i in range(NCORES)],
                          axis=0)
    return full, res


def kernel(**inputs):
    full, _ = _run(inputs)
    return full
